# revision 10
# baseline (speedup 1.0000x reference)
"""Trainium2 Bass kernel for nn_KVCacheMoE (B=8, S=2048, H=1024, E=8).

Strategy: batch-parallel across the 8 NeuronCores (core c owns batch c).
The router depends only on that batch's tokens, so every core computes its
own routing weights locally and its full output shard — no collectives.

Per core:
  phase A: stream x [S,H] fp32, cast bf16, DMA-xbar-transpose into
           SBUF-resident xT [H,S] bf16 (PE untouched).
  phase B: router: x_mean via DVE free-axis reduce over xT; fp32 PE matmuls
           relu(x_mean@Wr1+br1) -> softmax(h@Wr2+br2) -> r[8] -> R [128,8].
  phase C: per expert e: stream We[e] fp32, cast bf16 on ACT; 256 bf16
           matmuls (K=1024, N=512) accumulate x@We[e] in PSUM fp32;
           epilogue: DVE  mt = r_e*psum + r_e*be[e]  (scalar_tensor_tensor),
                     GpSimd acc += relu(mt)          (scalar_tensor_tensor).
  phase D: out DMA per tile as the last expert finishes it.

Engine/queue split: sync issues only xbar-transpose DMAs (keeps its DMA
queues in transpose mode); scalar issues all bulk copy DMAs + casts.
"""
import numpy as np
from contextlib import ExitStack

import concourse.bass as bass
import concourse.tile as tile
from concourse import bacc, mybir
from concourse.bass_utils import run_bass_kernel_spmd

B, S, H, E = 8, 2048, 1024, 8
N_CORES = 8
P = 128
NF = 512  # matmul moving free dim / PSUM bank width (fp32)
F32 = mybir.dt.float32
BF16 = mybir.dt.bfloat16
AX = mybir.AxisListType
ALU = mybir.AluOpType
ACTF = mybir.ActivationFunctionType


def build_nc(s=S):
    t_tiles = s // P
    h_tiles = H // P
    d_chunks = H // NF

    nc = bacc.Bacc("TRN2", target_bir_lowering=False, debug=False)
    x_ap = nc.dram_tensor("x", [s, H], F32, kind="ExternalInput").ap()
    we_ap = nc.dram_tensor("We", [E, H, H], F32, kind="ExternalInput").ap()
    be_ap = nc.dram_tensor("be", [E, H], F32, kind="ExternalInput").ap()
    wr1_ap = nc.dram_tensor("Wr1", [H, H], F32, kind="ExternalInput").ap()
    br1_ap = nc.dram_tensor("br1", [H], F32, kind="ExternalInput").ap()
    wr2_ap = nc.dram_tensor("Wr2", [H, E], F32, kind="ExternalInput").ap()
    br2_ap = nc.dram_tensor("br2", [E], F32, kind="ExternalInput").ap()
    out_ap = nc.dram_tensor("out", [s, H], F32, kind="ExternalOutput").ap()

    with tile.TileContext(nc) as tc, ExitStack() as ctx:
        xstage = ctx.enter_context(tc.tile_pool(name="xstage", bufs=3))
        xbpool = ctx.enter_context(tc.tile_pool(name="xbp", bufs=3))
        xtpool = ctx.enter_context(tc.tile_pool(name="xt", bufs=1))
        accpool = ctx.enter_context(tc.tile_pool(name="acc", bufs=1))
        wr1pool = ctx.enter_context(tc.tile_pool(name="wr1", bufs=4))
        wrawp = ctx.enter_context(tc.tile_pool(name="wraw", bufs=3))
        wqpool = ctx.enter_context(tc.tile_pool(name="wq", bufs=2))
        bepool = ctx.enter_context(tc.tile_pool(name="bep", bufs=2))
        mpool = ctx.enter_context(tc.tile_pool(name="mp", bufs=6))
        rpool = ctx.enter_context(tc.tile_pool(name="rp", bufs=1))
        ps = ctx.enter_context(tc.tile_pool(name="ps", bufs=8, space="PSUM"))

        ones_row = rpool.tile([1, P], F32, tag="ones_row")
        nc.vector.memset(ones_row, 1.0)

        # persistent SBUF residents
        xT = [xtpool.tile([P, s], BF16, tag=f"xT{j}", name=f"xT{j}") for j in range(h_tiles)]
        acc = [accpool.tile([P, H], F32, tag=f"acc{i}", name=f"acc{i}") for i in range(t_tiles)]

        # expert-0 weights first so the PE can start ASAP
        def load_wq(e):
            wq = wqpool.tile([P, h_tiles, H], BF16, tag="wq", name=f"wq{e}")
            for hj in range(h_tiles):
                wr = wrawp.tile([P, H], F32, tag="wr", name=f"wr{e}_{hj}")
                nc.scalar.dma_start(wr[:], we_ap[e, bass.ts(hj, P), :])
                nc.scalar.copy(wq[:, hj, :], wr[:])
            return wq

        wq0 = load_wq(0)

        # ---- phase A: load x, cast bf16, DMA-transpose into xT ----
        for ti in range(t_tiles):
            xs = xstage.tile([P, H], F32, tag="xs")
            nc.scalar.dma_start(xs[:], x_ap[bass.ts(ti, P), :])
            xb = xbpool.tile([P, H], BF16, tag="xb")
            nc.gpsimd.tensor_copy(xb[:], xs[:])
            for hj in range(h_tiles):
                nc.sync.dma_start_transpose(
                    xT[hj][:, bass.ts(ti, P)], xb[:, bass.ts(hj, P)]
                )

        # ---- phase B: router (fp32) ----
        xsum = rpool.tile([P, h_tiles], F32, tag="xsum")
        for hj in range(h_tiles):
            nc.vector.reduce_sum(xsum[:, hj : hj + 1], xT[hj][:], axis=AX.X)
        xmean = rpool.tile([P, h_tiles], F32, tag="xmean")
        nc.scalar.mul(xmean[:], xsum[:], 1.0 / s)

        hvec_ps = ps.tile([P, h_tiles], F32, tag="ps")
        for dj in range(h_tiles):
            for hj in range(h_tiles):
                w1t = wr1pool.tile([P, P], F32, tag="w1t")
                nc.scalar.dma_start(w1t[:], wr1_ap[bass.ts(hj, P), bass.ts(dj, P)])
                nc.tensor.matmul(
                    hvec_ps[:, dj : dj + 1],
                    w1t[:],
                    xmean[:, hj : hj + 1],
                    start=(hj == 0),
                    stop=(hj == h_tiles - 1),
                )
        br1t = rpool.tile([P, h_tiles], F32, tag="br1t")
        nc.scalar.dma_start(br1t[:], br1_ap.rearrange("(d p) -> p d", p=P))
        hsb = rpool.tile([P, h_tiles], F32, tag="hsb")
        nc.vector.tensor_add(hsb[:], hvec_ps[:], br1t[:])
        nc.vector.tensor_scalar_max(hsb[:], hsb[:], 0.0)

        lg_ps = ps.tile([1, E], F32, tag="ps")
        for dj in range(h_tiles):
            w2t = rpool.tile([P, E], F32, tag=f"w2t{dj}", name=f"w2t{dj}")
            nc.scalar.dma_start(w2t[:], wr2_ap[bass.ts(dj, P), :])
            nc.tensor.matmul(
                lg_ps[:],
                hsb[:, dj : dj + 1],
                w2t[:],
                start=(dj == 0),
                stop=(dj == h_tiles - 1),
            )
        br2t = rpool.tile([1, E], F32, tag="br2t")
        nc.scalar.dma_start(br2t[:], br2_ap.rearrange("(a e) -> a e", a=1))
        logits = rpool.tile([1, E], F32, tag="logits")
        nc.vector.tensor_add(logits[:], lg_ps[:], br2t[:])
        mx = rpool.tile([1, 1], F32, tag="mx")
        nc.vector.reduce_max(mx[:], logits[:], axis=AX.X)
        nmx = rpool.tile([1, 1], F32, tag="nmx")
        nc.vector.tensor_scalar_mul(nmx[:], mx[:], -1.0)
        ex = rpool.tile([1, E], F32, tag="ex")
        nc.scalar.activation(ex[:], logits[:], ACTF.Exp, bias=nmx[:], scale=1.0)
        sm = rpool.tile([1, 1], F32, tag="sm")
        nc.vector.reduce_sum(sm[:], ex[:], axis=AX.X)
        rinv = rpool.tile([1, 1], F32, tag="rinv")
        nc.vector.reciprocal(rinv[:], sm[:])
        rvec = rpool.tile([1, E], F32, tag="rvec")
        nc.vector.tensor_scalar_mul(rvec[:], ex[:], rinv[:])
        r_ps = ps.tile([P, E], F32, tag="ps")
        nc.tensor.matmul(r_ps[:], ones_row[:], rvec[:], start=True, stop=True)
        rsb = rpool.tile([P, E], F32, tag="rsb")
        nc.scalar.copy(rsb[:], r_ps[:])

        # ---- phase C: experts ----
        wq_cur = wq0
        for e in range(E):
            wq = wq_cur
            ber = bepool.tile([P, H], F32, tag="ber")
            nc.gpsimd.dma_start(ber[:], be_ap[e : e + 1, :].to_broadcast([P, H]))
            bep = bepool.tile([P, H], F32, tag="bep")
            nc.scalar.mul(bep[:], ber[:], rsb[:, e : e + 1])

            for ti in range(t_tiles):
                for dc in range(d_chunks):
                    mm_ps = ps.tile([P, NF], F32, tag="ps")
                    for hj in range(h_tiles):
                        nc.tensor.matmul(
                            mm_ps[:],
                            xT[hj][:, bass.ts(ti, P)],
                            wq[:, hj, bass.ts(dc, NF)],
                            start=(hj == 0),
                            stop=(hj == h_tiles - 1),
                        )
                    mt = mpool.tile([P, NF], F32, tag="mt")
                    # mt = r_e * psum + r_e*be  (r_e per-partition scalar)
                    nc.vector.scalar_tensor_tensor(
                        mt[:],
                        mm_ps[:],
                        rsb[:, e : e + 1],
                        bep[:, bass.ts(dc, NF)],
                        op0=ALU.mult,
                        op1=ALU.add,
                    )
                    aslice = acc[ti][:, bass.ts(dc, NF)]
                    if e == 0:
                        nc.vector.tensor_scalar_max(aslice, mt[:], 0.0)
                    else:
                        # acc += relu(mt)
                        nc.vector.scalar_tensor_tensor(
                            aslice, mt[:], 0.0, aslice, op0=ALU.max, op1=ALU.add
                        )
                if e == E - 1:
                    nc.scalar.dma_start(out_ap[bass.ts(ti, P), :], acc[ti][:])
                elif ti == 0:
                    wq_cur = load_wq(e + 1)

    nc.compile()
    return nc


_nc_cache = {}


def _get_nc(s):
    if s not in _nc_cache:
        _nc_cache[s] = build_nc(s)
    return _nc_cache[s]


def kernel(x, We, be, Wr1, br1, Wr2, br2):
    x = np.ascontiguousarray(np.asarray(x, dtype=np.float32))
    We = np.ascontiguousarray(np.asarray(We, dtype=np.float32))
    be = np.ascontiguousarray(np.asarray(be, dtype=np.float32))
    Wr1 = np.ascontiguousarray(np.asarray(Wr1, dtype=np.float32))
    br1 = np.ascontiguousarray(np.asarray(br1, dtype=np.float32))
    Wr2 = np.ascontiguousarray(np.asarray(Wr2, dtype=np.float32))
    br2 = np.ascontiguousarray(np.asarray(br2, dtype=np.float32))

    s = x.shape[1]
    nc = _get_nc(s)
    shared = {"We": We, "be": be, "Wr1": Wr1, "br1": br1, "Wr2": Wr2, "br2": br2}
    in_maps = [{"x": x[c], **shared} for c in range(N_CORES)]
    res = run_bass_kernel_spmd(nc, in_maps, list(range(N_CORES)))
    return np.stack([res.results[c]["out"] for c in range(N_CORES)], axis=0)


# revision 11
# speedup vs baseline: 1.4203x; 1.4203x over previous
"""Trainium2 Bass kernel for nn_KVCacheMoE (B=8, S=2048, H=1024, E=8).

Strategy: batch-parallel across the 8 NeuronCores (core c owns batch c).
The router depends only on that batch's tokens, so every core computes its
own routing weights locally and its full output shard — no collectives.

Per core:
  phase A: stream x [S,H] fp32, cast bf16, DMA-xbar-transpose into
           SBUF-resident xT [H,S] bf16 (PE untouched).
  phase B: router: x_mean via DVE free-axis reduce over xT; fp32 PE matmuls
           relu(x_mean@Wr1+br1) -> softmax(h@Wr2+br2) -> r[8] -> R [128,8].
  phase C: per expert e: stream We[e] fp32, cast bf16 on ACT; 256 bf16
           matmuls (K=1024, N=512) accumulate x@We[e] in PSUM fp32;
           epilogue: DVE  mt = r_e*psum + r_e*be[e]  (scalar_tensor_tensor),
                     GpSimd acc += relu(mt)          (scalar_tensor_tensor).
  phase D: out DMA per tile as the last expert finishes it.

Engine/queue split: sync issues only xbar-transpose DMAs (keeps its DMA
queues in transpose mode); scalar issues all bulk copy DMAs + casts.
"""
import numpy as np
from contextlib import ExitStack

import concourse.bass as bass
import concourse.tile as tile
from concourse import bacc, mybir
from concourse.bass_utils import run_bass_kernel_spmd
from concourse.masks import make_identity

B, S, H, E = 8, 2048, 1024, 8
N_CORES = 8
P = 128
NF = 512  # matmul moving free dim / PSUM bank width (fp32)
F32 = mybir.dt.float32
BF16 = mybir.dt.bfloat16
AX = mybir.AxisListType
ALU = mybir.AluOpType
ACTF = mybir.ActivationFunctionType


def build_nc(s=S):
    t_tiles = s // P
    h_tiles = H // P
    d_chunks = H // NF

    nc = bacc.Bacc("TRN2", target_bir_lowering=False, debug=False)
    x_ap = nc.dram_tensor("x", [s, H], F32, kind="ExternalInput").ap()
    we_ap = nc.dram_tensor("We", [E, H, H], F32, kind="ExternalInput").ap()
    be_ap = nc.dram_tensor("be", [E, H], F32, kind="ExternalInput").ap()
    wr1_ap = nc.dram_tensor("Wr1", [H, H], F32, kind="ExternalInput").ap()
    br1_ap = nc.dram_tensor("br1", [H], F32, kind="ExternalInput").ap()
    wr2_ap = nc.dram_tensor("Wr2", [H, E], F32, kind="ExternalInput").ap()
    br2_ap = nc.dram_tensor("br2", [E], F32, kind="ExternalInput").ap()
    out_ap = nc.dram_tensor("out", [s, H], F32, kind="ExternalOutput").ap()

    with tile.TileContext(nc) as tc, ExitStack() as ctx:
        xstage = ctx.enter_context(tc.tile_pool(name="xstage", bufs=3))
        xbpool = ctx.enter_context(tc.tile_pool(name="xbp", bufs=3))
        xtpool = ctx.enter_context(tc.tile_pool(name="xt", bufs=1))
        accpool = ctx.enter_context(tc.tile_pool(name="acc", bufs=1))
        wr1pool = ctx.enter_context(tc.tile_pool(name="wr1", bufs=4))
        wrawp = ctx.enter_context(tc.tile_pool(name="wraw", bufs=3))
        wqpool = ctx.enter_context(tc.tile_pool(name="wq", bufs=2))
        bepool = ctx.enter_context(tc.tile_pool(name="bep", bufs=2))
        mpool = ctx.enter_context(tc.tile_pool(name="mp", bufs=6))
        rpool = ctx.enter_context(tc.tile_pool(name="rp", bufs=1))
        ps = ctx.enter_context(tc.tile_pool(name="ps", bufs=8, space="PSUM"))

        ones_row = rpool.tile([1, P], F32, tag="ones_row")
        nc.vector.memset(ones_row, 1.0)

        # persistent SBUF residents
        xT = [xtpool.tile([P, s], BF16, tag=f"xT{j}", name=f"xT{j}") for j in range(h_tiles)]
        acc = [accpool.tile([P, H], F32, tag=f"acc{i}", name=f"acc{i}") for i in range(t_tiles)]

        # expert-0 weights first so the PE can start ASAP
        def load_wq(e):
            wq = wqpool.tile([P, h_tiles, H], BF16, tag="wq", name=f"wq{e}")
            for hj in range(h_tiles):
                wr = wrawp.tile([P, H], F32, tag="wr", name=f"wr{e}_{hj}")
                nc.sync.dma_start(wr[:], we_ap[e, bass.ts(hj, P), :])
                nc.scalar.copy(wq[:, hj, :], wr[:])
            return wq

        wq0 = load_wq(0)

        ident = rpool.tile([P, P], BF16, tag="ident")
        make_identity(nc, ident)

        # ---- phase A: load x, cast bf16, PE-transpose into xT ----
        for ti in range(t_tiles):
            xs = xstage.tile([P, H], F32, tag="xs")
            nc.sync.dma_start(xs[:], x_ap[bass.ts(ti, P), :])
            xb = xbpool.tile([P, H], BF16, tag="xb")
            nc.vector.tensor_copy(xb[:], xs[:])
            for hj in range(h_tiles):
                pt = ps.tile([P, P], BF16, tag="ps")
                nc.tensor.transpose(pt[:], xb[:, bass.ts(hj, P)], ident[:])
                if hj % 2 == 0:
                    nc.vector.tensor_copy(xT[hj][:, bass.ts(ti, P)], pt[:])
                else:
                    nc.scalar.copy(xT[hj][:, bass.ts(ti, P)], pt[:])

        # ---- phase B: router (fp32) ----
        xsum = rpool.tile([P, h_tiles], F32, tag="xsum")
        for hj in range(h_tiles):
            nc.vector.reduce_sum(xsum[:, hj : hj + 1], xT[hj][:], axis=AX.X)
        xmean = rpool.tile([P, h_tiles], F32, tag="xmean")
        nc.scalar.mul(xmean[:], xsum[:], 1.0 / s)

        hvec_ps = ps.tile([P, h_tiles], F32, tag="ps")
        for dj in range(h_tiles):
            for hj in range(h_tiles):
                w1t = wr1pool.tile([P, P], F32, tag="w1t")
                nc.sync.dma_start(w1t[:], wr1_ap[bass.ts(hj, P), bass.ts(dj, P)])
                nc.tensor.matmul(
                    hvec_ps[:, dj : dj + 1],
                    w1t[:],
                    xmean[:, hj : hj + 1],
                    start=(hj == 0),
                    stop=(hj == h_tiles - 1),
                )
        br1t = rpool.tile([P, h_tiles], F32, tag="br1t")
        nc.sync.dma_start(br1t[:], br1_ap.rearrange("(d p) -> p d", p=P))
        hsb = rpool.tile([P, h_tiles], F32, tag="hsb")
        nc.vector.tensor_add(hsb[:], hvec_ps[:], br1t[:])
        nc.vector.tensor_scalar_max(hsb[:], hsb[:], 0.0)

        lg_ps = ps.tile([1, E], F32, tag="ps")
        for dj in range(h_tiles):
            w2t = rpool.tile([P, E], F32, tag=f"w2t{dj}", name=f"w2t{dj}")
            nc.sync.dma_start(w2t[:], wr2_ap[bass.ts(dj, P), :])
            nc.tensor.matmul(
                lg_ps[:],
                hsb[:, dj : dj + 1],
                w2t[:],
                start=(dj == 0),
                stop=(dj == h_tiles - 1),
            )
        br2t = rpool.tile([1, E], F32, tag="br2t")
        nc.sync.dma_start(br2t[:], br2_ap.rearrange("(a e) -> a e", a=1))
        logits = rpool.tile([1, E], F32, tag="logits")
        nc.vector.tensor_add(logits[:], lg_ps[:], br2t[:])
        mx = rpool.tile([1, 1], F32, tag="mx")
        nc.vector.reduce_max(mx[:], logits[:], axis=AX.X)
        nmx = rpool.tile([1, 1], F32, tag="nmx")
        nc.vector.tensor_scalar_mul(nmx[:], mx[:], -1.0)
        ex = rpool.tile([1, E], F32, tag="ex")
        nc.scalar.activation(ex[:], logits[:], ACTF.Exp, bias=nmx[:], scale=1.0)
        sm = rpool.tile([1, 1], F32, tag="sm")
        nc.vector.reduce_sum(sm[:], ex[:], axis=AX.X)
        rinv = rpool.tile([1, 1], F32, tag="rinv")
        nc.vector.reciprocal(rinv[:], sm[:])
        rvec = rpool.tile([1, E], F32, tag="rvec")
        nc.vector.tensor_scalar_mul(rvec[:], ex[:], rinv[:])
        r_ps = ps.tile([P, E], F32, tag="ps")
        nc.tensor.matmul(r_ps[:], ones_row[:], rvec[:], start=True, stop=True)
        rsb = rpool.tile([P, E], F32, tag="rsb")
        nc.scalar.copy(rsb[:], r_ps[:])

        # ---- phase C: experts ----
        wq_cur = wq0
        for e in range(E):
            wq = wq_cur
            ber = bepool.tile([P, H], F32, tag="ber")
            nc.gpsimd.dma_start(ber[:], be_ap[e : e + 1, :].to_broadcast([P, H]))
            bep = bepool.tile([P, H], F32, tag="bep")
            nc.scalar.mul(bep[:], ber[:], rsb[:, e : e + 1])

            for ti in range(t_tiles):
                for dc in range(d_chunks):
                    mm_ps = ps.tile([P, NF], F32, tag="ps")
                    for hj in range(h_tiles):
                        nc.tensor.matmul(
                            mm_ps[:],
                            xT[hj][:, bass.ts(ti, P)],
                            wq[:, hj, bass.ts(dc, NF)],
                            start=(hj == 0),
                            stop=(hj == h_tiles - 1),
                        )
                    mt = mpool.tile([P, NF], F32, tag="mt")
                    # mt = r_e * psum + r_e*be  (r_e per-partition scalar)
                    nc.vector.scalar_tensor_tensor(
                        mt[:],
                        mm_ps[:],
                        rsb[:, e : e + 1],
                        bep[:, bass.ts(dc, NF)],
                        op0=ALU.mult,
                        op1=ALU.add,
                    )
                    aslice = acc[ti][:, bass.ts(dc, NF)]
                    if e == 0:
                        nc.vector.tensor_scalar_max(aslice, mt[:], 0.0)
                    else:
                        # acc += relu(mt)
                        nc.vector.scalar_tensor_tensor(
                            aslice, mt[:], 0.0, aslice, op0=ALU.max, op1=ALU.add
                        )
                if e == E - 1:
                    nc.sync.dma_start(out_ap[bass.ts(ti, P), :], acc[ti][:])
                elif ti == 0:
                    wq_cur = load_wq(e + 1)

    nc.compile()
    return nc


_nc_cache = {}


def _get_nc(s):
    if s not in _nc_cache:
        _nc_cache[s] = build_nc(s)
    return _nc_cache[s]


def kernel(x, We, be, Wr1, br1, Wr2, br2):
    x = np.ascontiguousarray(np.asarray(x, dtype=np.float32))
    We = np.ascontiguousarray(np.asarray(We, dtype=np.float32))
    be = np.ascontiguousarray(np.asarray(be, dtype=np.float32))
    Wr1 = np.ascontiguousarray(np.asarray(Wr1, dtype=np.float32))
    br1 = np.ascontiguousarray(np.asarray(br1, dtype=np.float32))
    Wr2 = np.ascontiguousarray(np.asarray(Wr2, dtype=np.float32))
    br2 = np.ascontiguousarray(np.asarray(br2, dtype=np.float32))

    s = x.shape[1]
    nc = _get_nc(s)
    shared = {"We": We, "be": be, "Wr1": Wr1, "br1": br1, "Wr2": Wr2, "br2": br2}
    in_maps = [{"x": x[c], **shared} for c in range(N_CORES)]
    res = run_bass_kernel_spmd(nc, in_maps, list(range(N_CORES)))
    return np.stack([res.results[c]["out"] for c in range(N_CORES)], axis=0)


# revision 13
# speedup vs baseline: 1.4306x; 1.0073x over previous
"""Trainium2 Bass kernel for nn_KVCacheMoE (B=8, S=2048, H=1024, E=8).

Strategy: batch-parallel across the 8 NeuronCores (core c owns batch c).
The router depends only on that batch's tokens, so every core computes its
own routing weights locally and its full output shard — no collectives.

Per core:
  phase A: stream x [S,H] fp32, cast bf16, DMA-xbar-transpose into
           SBUF-resident xT [H,S] bf16 (PE untouched).
  phase B: router: x_mean via DVE free-axis reduce over xT; fp32 PE matmuls
           relu(x_mean@Wr1+br1) -> softmax(h@Wr2+br2) -> r[8] -> R [128,8].
  phase C: per expert e: stream We[e] fp32, cast bf16 on ACT; 256 bf16
           matmuls (K=1024, N=512) accumulate x@We[e] in PSUM fp32;
           epilogue: DVE  mt = r_e*psum + r_e*be[e]  (scalar_tensor_tensor),
                     GpSimd acc += relu(mt)          (scalar_tensor_tensor).
  phase D: out DMA per tile as the last expert finishes it.

Engine/queue split: sync issues only xbar-transpose DMAs (keeps its DMA
queues in transpose mode); scalar issues all bulk copy DMAs + casts.
"""
import numpy as np
from contextlib import ExitStack

import concourse.bass as bass
import concourse.tile as tile
from concourse import bacc, mybir
from concourse.bass_utils import run_bass_kernel_spmd
from concourse.masks import make_identity

B, S, H, E = 8, 2048, 1024, 8
N_CORES = 8
P = 128
NF = 512  # matmul moving free dim / PSUM bank width (fp32)
F32 = mybir.dt.float32
BF16 = mybir.dt.bfloat16
AX = mybir.AxisListType
ALU = mybir.AluOpType
ACTF = mybir.ActivationFunctionType


def build_nc(s=S):
    t_tiles = s // P
    h_tiles = H // P
    d_chunks = H // NF

    nc = bacc.Bacc("TRN2", target_bir_lowering=False, debug=False)
    x_ap = nc.dram_tensor("x", [s, H], F32, kind="ExternalInput").ap()
    we_ap = nc.dram_tensor("We", [E, H, H], F32, kind="ExternalInput").ap()
    be_ap = nc.dram_tensor("be", [E, H], F32, kind="ExternalInput").ap()
    wr1_ap = nc.dram_tensor("Wr1", [H, H], F32, kind="ExternalInput").ap()
    br1_ap = nc.dram_tensor("br1", [H], F32, kind="ExternalInput").ap()
    wr2_ap = nc.dram_tensor("Wr2", [H, E], F32, kind="ExternalInput").ap()
    br2_ap = nc.dram_tensor("br2", [E], F32, kind="ExternalInput").ap()
    out_ap = nc.dram_tensor("out", [s, H], F32, kind="ExternalOutput").ap()

    with tile.TileContext(nc) as tc, ExitStack() as ctx:
        xstage = ctx.enter_context(tc.tile_pool(name="xstage", bufs=3))
        xbpool = ctx.enter_context(tc.tile_pool(name="xbp", bufs=3))
        xtpool = ctx.enter_context(tc.tile_pool(name="xt", bufs=1))
        accpool = ctx.enter_context(tc.tile_pool(name="acc", bufs=1))
        wr1pool = ctx.enter_context(tc.tile_pool(name="wr1", bufs=4))
        wrawp = ctx.enter_context(tc.tile_pool(name="wraw", bufs=3))
        wqpool = ctx.enter_context(tc.tile_pool(name="wq", bufs=2))
        bepool = ctx.enter_context(tc.tile_pool(name="bep", bufs=2))
        mpool = ctx.enter_context(tc.tile_pool(name="mp", bufs=6))
        rpool = ctx.enter_context(tc.tile_pool(name="rp", bufs=1))
        ps = ctx.enter_context(tc.tile_pool(name="ps", bufs=8, space="PSUM"))

        ones_row = rpool.tile([1, P], F32, tag="ones_row")
        nc.vector.memset(ones_row, 1.0)

        # persistent SBUF residents
        xT = [xtpool.tile([P, s], BF16, tag=f"xT{j}", name=f"xT{j}") for j in range(h_tiles)]
        acc = [accpool.tile([P, H], F32, tag=f"acc{i}", name=f"acc{i}") for i in range(t_tiles)]

        # expert-0 weights first so the PE can start ASAP
        def load_wq(e):
            wq = wqpool.tile([P, h_tiles, H], BF16, tag="wq", name=f"wq{e}")
            for hj in range(h_tiles):
                wr = wrawp.tile([P, H], F32, tag="wr", name=f"wr{e}_{hj}")
                nc.sync.dma_start(wr[:], we_ap[e, bass.ts(hj, P), :])
                nc.scalar.copy(wq[:, hj, :], wr[:])
            return wq

        ident = rpool.tile([P, P], BF16, tag="ident")
        make_identity(nc, ident)

        # expert-0 bias broadcast (router-independent)
        ber0 = bepool.tile([P, H], F32, tag="ber", name="ber0")
        nc.gpsimd.dma_start(ber0[:], be_ap[0:1, :].to_broadcast([P, H]))

        def mm_group(wq, ti, dc):
            mm_ps = ps.tile([P, NF], F32, tag="ps", name=f"mmps")
            for hj in range(h_tiles):
                nc.tensor.matmul(
                    mm_ps[:],
                    xT[hj][:, bass.ts(ti, P)],
                    wq[:, hj, bass.ts(dc, NF)],
                    start=(hj == 0),
                    stop=(hj == h_tiles - 1),
                )
            return mm_ps

        # ---- phase A interleaved with expert 0 ----
        # e0 epilogue is unscaled (acc = relu(y0+be0)); rescaled by r0 in e1.
        wq0 = None
        for ti in range(t_tiles):
            xs = xstage.tile([P, H], F32, tag="xs")
            nc.sync.dma_start(xs[:], x_ap[bass.ts(ti, P), :])
            xb = xbpool.tile([P, H], BF16, tag="xb")
            nc.vector.tensor_copy(xb[:], xs[:])
            for hj in range(h_tiles):
                pt = ps.tile([P, P], BF16, tag="ps")
                nc.tensor.transpose(pt[:], xb[:, bass.ts(hj, P)], ident[:])
                if hj % 2 == 0:
                    nc.vector.tensor_copy(xT[hj][:, bass.ts(ti, P)], pt[:])
                else:
                    nc.scalar.copy(xT[hj][:, bass.ts(ti, P)], pt[:])
            if ti == 0:
                wq0 = load_wq(0)
            for dc in range(d_chunks):
                mm_ps = mm_group(wq0, ti, dc)
                mt = mpool.tile([P, NF], F32, tag="mt")
                nc.vector.tensor_add(mt[:], mm_ps[:], ber0[:, bass.ts(dc, NF)])
                nc.vector.tensor_scalar_max(acc[ti][:, bass.ts(dc, NF)], mt[:], 0.0)
            if ti == 0:
                wq1 = load_wq(1)

        # ---- phase B: router (fp32) ----
        xsum = rpool.tile([P, h_tiles], F32, tag="xsum")
        for hj in range(h_tiles):
            nc.vector.reduce_sum(xsum[:, hj : hj + 1], xT[hj][:], axis=AX.X)
        xmean = rpool.tile([P, h_tiles], F32, tag="xmean")
        nc.scalar.mul(xmean[:], xsum[:], 1.0 / s)

        hvec_ps = ps.tile([P, h_tiles], F32, tag="ps")
        for dj in range(h_tiles):
            for hj in range(h_tiles):
                w1t = wr1pool.tile([P, P], F32, tag="w1t")
                nc.sync.dma_start(w1t[:], wr1_ap[bass.ts(hj, P), bass.ts(dj, P)])
                nc.tensor.matmul(
                    hvec_ps[:, dj : dj + 1],
                    w1t[:],
                    xmean[:, hj : hj + 1],
                    start=(hj == 0),
                    stop=(hj == h_tiles - 1),
                )
        br1t = rpool.tile([P, h_tiles], F32, tag="br1t")
        nc.sync.dma_start(br1t[:], br1_ap.rearrange("(d p) -> p d", p=P))
        hsb = rpool.tile([P, h_tiles], F32, tag="hsb")
        nc.vector.tensor_add(hsb[:], hvec_ps[:], br1t[:])
        nc.vector.tensor_scalar_max(hsb[:], hsb[:], 0.0)

        lg_ps = ps.tile([1, E], F32, tag="ps")
        for dj in range(h_tiles):
            w2t = rpool.tile([P, E], F32, tag=f"w2t{dj}", name=f"w2t{dj}")
            nc.sync.dma_start(w2t[:], wr2_ap[bass.ts(dj, P), :])
            nc.tensor.matmul(
                lg_ps[:],
                hsb[:, dj : dj + 1],
                w2t[:],
                start=(dj == 0),
                stop=(dj == h_tiles - 1),
            )
        br2t = rpool.tile([1, E], F32, tag="br2t")
        nc.sync.dma_start(br2t[:], br2_ap.rearrange("(a e) -> a e", a=1))
        logits = rpool.tile([1, E], F32, tag="logits")
        nc.vector.tensor_add(logits[:], lg_ps[:], br2t[:])
        mx = rpool.tile([1, 1], F32, tag="mx")
        nc.vector.reduce_max(mx[:], logits[:], axis=AX.X)
        nmx = rpool.tile([1, 1], F32, tag="nmx")
        nc.vector.tensor_scalar_mul(nmx[:], mx[:], -1.0)
        ex = rpool.tile([1, E], F32, tag="ex")
        nc.scalar.activation(ex[:], logits[:], ACTF.Exp, bias=nmx[:], scale=1.0)
        sm = rpool.tile([1, 1], F32, tag="sm")
        nc.vector.reduce_sum(sm[:], ex[:], axis=AX.X)
        rinv = rpool.tile([1, 1], F32, tag="rinv")
        nc.vector.reciprocal(rinv[:], sm[:])
        rvec = rpool.tile([1, E], F32, tag="rvec")
        nc.vector.tensor_scalar_mul(rvec[:], ex[:], rinv[:])
        r_ps = ps.tile([P, E], F32, tag="ps")
        nc.tensor.matmul(r_ps[:], ones_row[:], rvec[:], start=True, stop=True)
        rsb = rpool.tile([P, E], F32, tag="rsb")
        nc.scalar.copy(rsb[:], r_ps[:])

        # ---- phase C: experts 1..7 ----
        wq_cur = wq1
        for e in range(1, E):
            wq = wq_cur
            ber = bepool.tile([P, H], F32, tag="ber")
            nc.gpsimd.dma_start(ber[:], be_ap[e : e + 1, :].to_broadcast([P, H]))
            bep = bepool.tile([P, H], F32, tag="bep")
            nc.scalar.mul(bep[:], ber[:], rsb[:, e : e + 1])

            for ti in range(t_tiles):
                if e == 1:
                    # apply expert-0 routing weight deferred from phase A
                    nc.vector.tensor_scalar_mul(acc[ti][:], acc[ti][:], rsb[:, 0:1])
                for dc in range(d_chunks):
                    mm_ps = mm_group(wq, ti, dc)
                    mt = mpool.tile([P, NF], F32, tag="mt")
                    # mt = r_e * psum + r_e*be  (r_e per-partition scalar)
                    nc.vector.scalar_tensor_tensor(
                        mt[:],
                        mm_ps[:],
                        rsb[:, e : e + 1],
                        bep[:, bass.ts(dc, NF)],
                        op0=ALU.mult,
                        op1=ALU.add,
                    )
                    aslice = acc[ti][:, bass.ts(dc, NF)]
                    # acc += relu(mt)
                    nc.vector.scalar_tensor_tensor(
                        aslice, mt[:], 0.0, aslice, op0=ALU.max, op1=ALU.add
                    )
                if e == E - 1:
                    nc.sync.dma_start(out_ap[bass.ts(ti, P), :], acc[ti][:])
                elif ti == 0 and e + 1 < E:
                    wq_cur = load_wq(e + 1)

    nc.compile()
    return nc


_nc_cache = {}


def _get_nc(s):
    if s not in _nc_cache:
        _nc_cache[s] = build_nc(s)
    return _nc_cache[s]


def kernel(x, We, be, Wr1, br1, Wr2, br2):
    x = np.ascontiguousarray(np.asarray(x, dtype=np.float32))
    We = np.ascontiguousarray(np.asarray(We, dtype=np.float32))
    be = np.ascontiguousarray(np.asarray(be, dtype=np.float32))
    Wr1 = np.ascontiguousarray(np.asarray(Wr1, dtype=np.float32))
    br1 = np.ascontiguousarray(np.asarray(br1, dtype=np.float32))
    Wr2 = np.ascontiguousarray(np.asarray(Wr2, dtype=np.float32))
    br2 = np.ascontiguousarray(np.asarray(br2, dtype=np.float32))

    s = x.shape[1]
    nc = _get_nc(s)
    shared = {"We": We, "be": be, "Wr1": Wr1, "br1": br1, "Wr2": Wr2, "br2": br2}
    in_maps = [{"x": x[c], **shared} for c in range(N_CORES)]
    res = run_bass_kernel_spmd(nc, in_maps, list(range(N_CORES)))
    return np.stack([res.results[c]["out"] for c in range(N_CORES)], axis=0)
